# revision 8
# baseline (speedup 1.0000x reference)
"""Trainium2 Bass kernel for MetaGraphMeanLayer (GNN message passing).

Strategy (edge-parallel, sort-based scatter, no collectives):
  - Host sorts edges by destination node (row). Nodes are split into 128-node
    blocks; each of the 8 cores owns a contiguous range of blocks (disjoint),
    so no cross-core reduction is needed.
  - Each block's edge list is padded to a uniform K tiles of 128 edges so the
    SPMD program structure is identical across cores; all per-core data
    differences flow through input tensors.
  - Device, per block:
      * load the block's x rows (node-major), transpose once on PE,
        pb = x_blk @ ew1[:128] — the x[row] term of the edge-MLP L1 enters
        via a one-hot matmul instead of a second gather
      * per 128-edge tile: indirect-gather x[col] rows, build one-hot
        sel[e, n] = (row_local[e] == n) with is_equal against an iota const
      * MLPs run channel-major ([ch, edges]); weights are matmul lhsT,
        L1 biases are per-partition activation biases fused into the ReLU
      * node-MLP L1's new_edge_attr term is host-fused:
        Wfuse = ew2 @ nw1[128:], bfuse = nw1[128:]^T @ eb2 (added to nb1)
      * scatter_sum = sum over tiles of sel^T @ msg, accumulated in PSUM per
        block, one [128 nodes, 128 ch] store per block
  - L2 biases are applied on host: new_edge_attr += eb2;
    new_x = (seg + cnt*nb2) / max(cnt, 1).
"""

import math
import sys
from contextlib import ExitStack

import numpy as np

sys.path.insert(0, "/opt/trn_rl_repo")

import ml_dtypes  # noqa: E402

import concourse.bass as bass  # noqa: E402
import concourse.mybir as mybir  # noqa: E402
import concourse.tile as tile  # noqa: E402
from concourse.masks import make_identity  # noqa: E402

P = 128
F32 = mybir.dt.float32
BF16 = mybir.dt.bfloat16
I32 = mybir.dt.int32



# ---------------------------------------------------------------------------
# Workaround: the walrus build in this container accepts at most ONE attached
# sem wait per instruction ("Too many sync wait commands").  Tile attaches
# several (notably on its tail Drain).  Split extras into standalone
# EventSemaphore instructions (the exact form nc.sync.wait_ge() emits).
def _split_multiwaits_json(bir_json: bytes) -> bytes:
    import json as _json

    j = _json.loads(bir_json)
    n_new = 0
    for fn in j.get("functions", []):
        for bb in fn.get("blocks", []):
            out = []
            for ins in bb.get("instructions", []):
                si = ins.get("sync_info") or {}
                waits = si.get("on_wait") or []
                if len(waits) > 1:
                    for w in waits[:-1]:
                        n_new += 1
                        out.append(
                            dict(
                                debug=ins.get("debug", 0),
                                engine=ins["engine"],
                                ins=[],
                                outs=[],
                                name=f"{ins['name']}-hoistw{n_new}",
                                opcode="EventSemaphore",
                                sync_info=dict(on_update=[], on_wait=[w]),
                            )
                        )
                    si["on_wait"] = [waits[-1]]
                out.append(ins)
            bb["instructions"] = out
    return _json.dumps(j).encode()


def _install_compile_patch():
    from concourse import bass_utils as _bu

    if getattr(_bu, "_multiwait_patch", False):
        return
    orig = _bu.compile_bir_kernel

    def patched(bir_json, tmpdir, neff_name="file.neff"):
        return orig(_split_multiwaits_json(bir_json), tmpdir, neff_name)

    _bu.compile_bir_kernel = patched
    _bu._multiwait_patch = True
    try:
        from concourse import bass2jax as _b2j

        _b2j.compile_bir_kernel = patched
    except Exception:
        pass


class Cfg:
    def __init__(self, n_nodes, n_edges, n_cores, k_tiles):
        self.n_nodes = n_nodes
        self.n_edges = n_edges
        self.n_cores = n_cores
        self.k_tiles = k_tiles  # tiles of 128 edge-slots per node block
        self.n_blocks = math.ceil(n_nodes / P)  # real node blocks
        self.bpc = math.ceil(self.n_blocks / n_cores)  # blocks per core
        self.slots_per_block = k_tiles * P
        # chunk split of a block's K*128 slots into matmul-N-sized pieces
        self.chunks = []
        off = 0
        rem = self.slots_per_block
        while rem > 0:
            cs = min(512, rem)
            self.chunks.append((off, cs))
            off += cs
            rem -= cs


def preprocess(x, edge_index, edge_attr, cfg):
    """Sort/pad edges; build per-core device input arrays."""
    row = np.asarray(edge_index[0], dtype=np.int64)
    col = np.asarray(edge_index[1], dtype=np.int64)
    order = np.argsort(row, kind="stable")
    row_s = row[order]
    bounds = np.searchsorted(row_s, P * np.arange(cfg.n_blocks + 1))

    S = cfg.slots_per_block
    n_total_blocks = cfg.n_cores * cfg.bpc
    slot_eid = np.zeros((n_total_blocks, S), dtype=np.int64)
    slot_valid = np.zeros((n_total_blocks, S), dtype=bool)
    slot_rowlocal = np.full((n_total_blocks, S), -1.0, dtype=np.float32)
    slot_col = np.zeros((n_total_blocks, S), dtype=np.int32)
    for b in range(cfg.n_blocks):
        s, e = bounds[b], bounds[b + 1]
        n = e - s
        assert n <= S, f"block {b} has {n} edges > {S} slots; raise k_tiles"
        eids = order[s:e]
        slot_eid[b, :n] = eids
        slot_valid[b, :n] = True
        slot_rowlocal[b, :n] = (row[eids] - P * b).astype(np.float32)
        slot_col[b, :n] = col[eids].astype(np.int32)

    x_bf = x.astype(ml_dtypes.bfloat16)
    pad_rows = n_total_blocks * P - x_bf.shape[0]
    x_pad = np.concatenate(
        [x_bf, np.zeros((pad_rows, x.shape[1]), dtype=ml_dtypes.bfloat16)], axis=0
    )

    per_core = []
    for c in range(cfg.n_cores):
        blks = slice(c * cfg.bpc, (c + 1) * cfg.bpc)
        ea = np.zeros((cfg.bpc, S, x.shape[1]), dtype=ml_dtypes.bfloat16)
        eid_c = slot_eid[blks]
        val_c = slot_valid[blks]
        ea[val_c] = edge_attr[eid_c[val_c]].astype(ml_dtypes.bfloat16)
        ea_cm = np.ascontiguousarray(ea.transpose(0, 2, 1))
        ci = slot_col[blks].reshape(cfg.bpc, cfg.k_tiles, P).transpose(0, 2, 1)
        rl = slot_rowlocal[blks].reshape(cfg.bpc, cfg.k_tiles, P).transpose(0, 2, 1)
        per_core.append(
            dict(
                ea_cm=ea_cm,
                colidx=np.ascontiguousarray(ci),
                rowlocal=np.ascontiguousarray(rl),
                x_full=x_pad,
                x_slice=np.ascontiguousarray(
                    x_pad[c * cfg.bpc * P : (c + 1) * cfg.bpc * P]
                ),
            )
        )
    meta = dict(slot_eid=slot_eid, slot_valid=slot_valid, row=row)
    return per_core, meta


def make_weight_inputs(ew1, eb1, ew2, eb2, nw1, nb1, nw2, nb2):
    ew1 = np.asarray(ew1, np.float32)
    eb1 = np.asarray(eb1, np.float32)
    ew2 = np.asarray(ew2, np.float32)
    eb2 = np.asarray(eb2, np.float32)
    nw1 = np.asarray(nw1, np.float32)
    nb1 = np.asarray(nb1, np.float32)
    nw2 = np.asarray(nw2, np.float32)
    bf = lambda a: np.ascontiguousarray(a.astype(ml_dtypes.bfloat16))
    f32 = lambda a: np.ascontiguousarray(a.astype(np.float32))
    wfuse = ew2 @ nw1[128:256]  # new_edge_attr -> h3, fused through L2e
    bfuse = eb2 @ nw1[128:256]  # eb2's contribution to h3 preactivation
    return dict(
        w1a=bf(ew1[:128]),
        w1b=bf(ew1[128:256]),
        w1c=bf(ew1[256:384]),
        w2e=bf(ew2),
        w1na=bf(nw1[:128]),
        wfuse=bf(wfuse),
        w2n=bf(nw2),
        b1e=f32(eb1[:, None]),
        b1nf=f32((nb1 + bfuse)[:, None]),
        iota_f=f32(np.tile(np.arange(P, dtype=np.float32)[None, :], (P, 1))),
    )


WNAMES = ["w1a", "w1b", "w1c", "w2e", "w1na", "wfuse", "w2n"]
BSHAPES = dict(b1e=[P, 1], b1nf=[P, 1], iota_f=[P, P])


def build_program(cfg):
    CH = P
    S = cfg.slots_per_block
    K = cfg.k_tiles
    nc = bass.Bass(
        "TRN2", target_bir_lowering=False, debug=False, num_devices=cfg.n_cores
    )

    d_x_full = nc.declare_dram_parameter(
        "x_full", [cfg.n_cores * cfg.bpc * P, CH], BF16, isOutput=False
    )
    d_x_slice = nc.declare_dram_parameter(
        "x_slice", [cfg.bpc * P, CH], BF16, isOutput=False
    )
    d_ea = nc.declare_dram_parameter("ea_cm", [cfg.bpc, CH, S], BF16, isOutput=False)
    d_ci = nc.declare_dram_parameter("colidx", [cfg.bpc, P, K], I32, isOutput=False)
    d_rl = nc.declare_dram_parameter("rowlocal", [cfg.bpc, P, K], F32, isOutput=False)
    dW = {k: nc.declare_dram_parameter(k, [CH, CH], BF16, isOutput=False) for k in WNAMES}
    dB = {k: nc.declare_dram_parameter(k, v, F32, isOutput=False) for k, v in BSHAPES.items()}
    d_ne = nc.declare_dram_parameter("ne_out", [cfg.bpc, S, CH], F32, isOutput=True)
    d_seg = nc.declare_dram_parameter("seg_out", [cfg.bpc, P, CH], F32, isOutput=True)

    relu = mybir.ActivationFunctionType.Relu

    with tile.TileContext(nc) as tc, ExitStack() as ctx:
        cpool = ctx.enter_context(tc.tile_pool(name="const", bufs=1))
        blkpool = ctx.enter_context(tc.tile_pool(name="blk", bufs=2))
        work = ctx.enter_context(tc.tile_pool(name="work", bufs=3))
        mpsum = ctx.enter_context(tc.tile_pool(name="mpsum", bufs=3, space="PSUM"))
        tpsum = ctx.enter_context(tc.tile_pool(name="tpsum", bufs=2, space="PSUM"))
        segpsum = ctx.enter_context(tc.tile_pool(name="segpsum", bufs=2, space="PSUM"))

        identb = cpool.tile([P, P], BF16)
        make_identity(nc, identb[:])

        W = {}
        for k in WNAMES:
            W[k] = cpool.tile([CH, CH], BF16, tag=f"W{k}", name=f"W_{k}")
            nc.sync.dma_start(out=W[k][:], in_=dW[k][:])
        B = {}
        for k, shp in BSHAPES.items():
            B[k] = cpool.tile(shp, F32, tag=f"B{k}", name=f"B_{k}")
            nc.sync.dma_start(out=B[k][:], in_=dB[k][:])

        for lb in range(cfg.bpc):
            x_blk = blkpool.tile([P, CH], BF16, tag="x_blk")
            nc.sync.dma_start(out=x_blk[:], in_=d_x_slice[lb * P : (lb + 1) * P, :])
            ci_blk = blkpool.tile([P, K], I32, tag="ci")
            nc.sync.dma_start(out=ci_blk[:], in_=d_ci[lb])
            rl_blk = blkpool.tile([P, K], F32, tag="rl")
            nc.sync.dma_start(out=rl_blk[:], in_=d_rl[lb])

            xbt_ps = tpsum.tile([P, 512], BF16, tag="tpb")
            nc.tensor.transpose(out=xbt_ps[:, :CH], in_=x_blk[:], identity=identb[:])
            x_blkT = blkpool.tile([CH, P], BF16, tag="xbt")
            nc.vector.tensor_copy(x_blkT[:], xbt_ps[:, :CH])
            pb_ps = mpsum.tile([P, 512], F32, tag="mm")
            nc.tensor.matmul(
                out=pb_ps[:, :CH], lhsT=x_blkT[:], rhs=W["w1a"][:], start=True, stop=True
            )
            pb = blkpool.tile([P, CH], BF16, tag="pb")
            nc.vector.tensor_copy(pb[:], pb_ps[:, :CH])

            seg_ps = segpsum.tile([P, CH], F32, tag="seg")

            for (coff, cs) in cfg.chunks:
                nt = cs // P
                t0 = coff // P
                ea_t = work.tile([CH, 512], BF16, tag="ea")
                nc.sync.dma_start(out=ea_t[:, :cs], in_=d_ea[lb, :, coff : coff + cs])

                sel_em = work.tile([P, 512], BF16, tag="sel_em")  # [e, n]
                xct_ps = tpsum.tile([P, 512], BF16, tag="tpb")
                selg_ps = tpsum.tile([P, 512], BF16, tag="tpb")
                for t in range(nt):
                    ts_ = slice(t * P, (t + 1) * P)
                    xc_em = work.tile([P, CH], BF16, tag="xc_em")
                    nc.gpsimd.indirect_dma_start(
                        out=xc_em[:],
                        out_offset=None,
                        in_=d_x_full[:],
                        in_offset=bass.IndirectOffsetOnAxis(
                            ap=ci_blk[:, t0 + t : t0 + t + 1], axis=0
                        ),
                    )
                    nc.tensor.transpose(
                        out=xct_ps[:, ts_], in_=xc_em[:], identity=identb[:]
                    )
                    nc.vector.tensor_tensor(
                        out=sel_em[:, ts_],
                        in0=rl_blk[:, t0 + t : t0 + t + 1].to_broadcast([P, P]),
                        in1=B["iota_f"][:],
                        op=mybir.AluOpType.is_equal,
                    )
                    nc.tensor.transpose(
                        out=selg_ps[:, ts_], in_=sel_em[:, ts_], identity=identb[:]
                    )
                xc_cm = work.tile([CH, 512], BF16, tag="xc_cm")
                nc.scalar.activation(
                    out=xc_cm[:, :cs],
                    in_=xct_ps[:, :cs],
                    func=mybir.ActivationFunctionType.Copy,
                )
                sel_g = work.tile([P, 512], BF16, tag="sel_g")  # [n, e]
                nc.vector.tensor_copy(sel_g[:, :cs], selg_ps[:, :cs])

                # ---- edge MLP (channel-major) ----
                h1_ps = mpsum.tile([CH, 512], F32, tag="mm")
                nc.tensor.matmul(
                    out=h1_ps[:, :cs], lhsT=pb[:], rhs=sel_g[:, :cs],
                    start=True, stop=False,
                )
                nc.tensor.matmul(
                    out=h1_ps[:, :cs], lhsT=W["w1b"][:], rhs=xc_cm[:, :cs],
                    start=False, stop=False,
                )
                nc.tensor.matmul(
                    out=h1_ps[:, :cs], lhsT=W["w1c"][:], rhs=ea_t[:, :cs],
                    start=False, stop=True,
                )
                h1s = work.tile([CH, 512], BF16, tag="h1s")
                nc.scalar.activation(
                    out=h1s[:, :cs], in_=h1_ps[:, :cs], func=relu, bias=B["b1e"][:]
                )

                # edge-major new_edge_attr (bias applied on host)
                ne_ps = mpsum.tile([P, 512], F32, tag="mm")
                for t in range(nt):
                    ts_ = slice(t * P, (t + 1) * P)
                    nc.tensor.matmul(
                        out=ne_ps[:, ts_], lhsT=h1s[:, ts_], rhs=W["w2e"][:],
                        start=True, stop=True,
                    )
                ne_sb = work.tile([P, 512], F32, tag="ne_sb")
                nc.vector.tensor_copy(ne_sb[:, :cs], ne_ps[:, :cs])
                for t in range(nt):
                    nc.sync.dma_start(
                        out=d_ne[lb, coff + t * P : coff + (t + 1) * P, :],
                        in_=ne_sb[:, t * CH : (t + 1) * CH],
                    )

                # ---- node MLP ----
                h3_ps = mpsum.tile([CH, 512], F32, tag="mm")
                nc.tensor.matmul(
                    out=h3_ps[:, :cs], lhsT=W["w1na"][:], rhs=xc_cm[:, :cs],
                    start=True, stop=False,
                )
                nc.tensor.matmul(
                    out=h3_ps[:, :cs], lhsT=W["wfuse"][:], rhs=h1s[:, :cs],
                    start=False, stop=True,
                )
                h3s = work.tile([CH, 512], BF16, tag="h3s")
                nc.scalar.activation(
                    out=h3s[:, :cs], in_=h3_ps[:, :cs], func=relu, bias=B["b1nf"][:]
                )

                msg_ps = mpsum.tile([P, 512], F32, tag="mm")
                for t in range(nt):
                    ts_ = slice(t * P, (t + 1) * P)
                    nc.tensor.matmul(
                        out=msg_ps[:, ts_], lhsT=h3s[:, ts_], rhs=W["w2n"][:],
                        start=True, stop=True,
                    )
                msg_em = work.tile([P, 512], BF16, tag="msg_em")
                nc.vector.tensor_copy(msg_em[:, :cs], msg_ps[:, :cs])

                # ---- scatter: seg += sel^T @ msg over the block ----
                for t in range(nt):
                    ts_ = slice(t * P, (t + 1) * P)
                    nc.tensor.matmul(
                        out=seg_ps[:],
                        lhsT=sel_em[:, ts_],
                        rhs=msg_em[:, ts_],
                        start=(t0 + t == 0),
                        stop=(t0 + t == K - 1),
                        skip_group_check=True,
                    )

            seg_sb = blkpool.tile([P, CH], F32, tag="seg_sb")
            nc.vector.tensor_copy(seg_sb[:], seg_ps[:])
            nc.sync.dma_start(out=d_seg[lb], in_=seg_sb[:])

    return nc


def postprocess(results, meta, cfg, eb2, nb2):
    n_nodes, n_edges = cfg.n_nodes, cfg.n_edges
    ch = P
    row = meta["row"]
    cnt = np.bincount(row, minlength=n_nodes).astype(np.float32)
    denom = np.maximum(cnt, 1.0)
    seg = np.concatenate(
        [np.asarray(r["seg_out"], dtype=np.float32).reshape(-1, ch) for r in results],
        axis=0,
    )
    new_x = (seg[:n_nodes] + cnt[:, None] * np.asarray(nb2, np.float32)[None, :]) / denom[
        :, None
    ]
    new_ea = np.empty((n_edges, ch), dtype=np.float32)
    sv = meta["slot_valid"].reshape(cfg.n_cores, -1)
    se = meta["slot_eid"].reshape(cfg.n_cores, -1)
    for c, r in enumerate(results):
        ne = np.asarray(r["ne_out"], dtype=np.float32).reshape(-1, ch)
        v = sv[c]
        new_ea[se[c][v]] = ne[v]
    new_ea += np.asarray(eb2, np.float32)[None, :]
    return new_x.astype(np.float32), new_ea


def build_all(x, edge_index, edge_attr, ew1, eb1, ew2, eb2, nw1, nb1, nw2, nb2,
              n_cores=8):
    x = np.asarray(x, dtype=np.float32)
    edge_attr = np.asarray(edge_attr, dtype=np.float32)
    n_nodes = x.shape[0]
    n_edges = edge_attr.shape[0]
    row = np.asarray(edge_index[0], dtype=np.int64)
    blk_cnt = np.bincount(row // P, minlength=math.ceil(n_nodes / P))
    k_tiles = max(1, int(math.ceil(blk_cnt.max() / P)))
    cfg = Cfg(n_nodes, n_edges, n_cores, k_tiles)

    per_core, meta = preprocess(x, edge_index, edge_attr, cfg)
    wdict = make_weight_inputs(ew1, eb1, ew2, eb2, nw1, nb1, nw2, nb2)
    in_maps = []
    for c in range(n_cores):
        m = dict(per_core[c])
        m.update(wdict)
        in_maps.append(m)
    nc = build_program(cfg)
    return nc, in_maps, meta, cfg


def kernel(x, edge_index, edge_attr, ew1, eb1, ew2, eb2, nw1, nb1, nw2, nb2):
    _install_compile_patch()
    from concourse.bass_utils import run_bass_kernel_spmd

    nc, in_maps, meta, cfg = build_all(
        x, edge_index, edge_attr, ew1, eb1, ew2, eb2, nw1, nb1, nw2, nb2, n_cores=8
    )
    res = run_bass_kernel_spmd(nc, in_maps, list(range(cfg.n_cores)))
    return postprocess(res.results, meta, cfg, eb2, nb2)


# revision 18
# speedup vs baseline: 67.4798x; 67.4798x over previous
"""Trainium2 Bass kernel for MetaGraphMeanLayer (GNN message passing).

Strategy (edge-parallel, sort-based scatter, no collectives):
  - Host sorts edges by destination node (row). Nodes are split into 128-node
    blocks; each of the 8 cores owns a contiguous range of blocks (disjoint),
    so no cross-core reduction is needed.
  - Each block's edge list is padded to a uniform K tiles of 128 edges so the
    SPMD program structure is identical across cores; all per-core data
    differences flow through input tensors.
  - Device, per block:
      * load the block's x rows (node-major), transpose once on PE,
        pb = x_blk @ ew1[:128] — the x[row] term of the edge-MLP L1 enters
        via a one-hot matmul instead of a second gather
      * per 128-edge tile: indirect-gather x[col] rows, build one-hot
        sel[e, n] = (row_local[e] == n) with is_equal against an iota const
      * MLPs run channel-major ([ch, edges]); weights are matmul lhsT,
        L1 biases are per-partition activation biases fused into the ReLU
      * node-MLP L1's new_edge_attr term is host-fused:
        Wfuse = ew2 @ nw1[128:], bfuse = nw1[128:]^T @ eb2 (added to nb1)
      * scatter_sum = sum over tiles of sel^T @ msg, accumulated in PSUM per
        block, one [128 nodes, 128 ch] store per block
  - L2 biases are applied on host: new_edge_attr += eb2;
    new_x = (seg + cnt*nb2) / max(cnt, 1).
"""

import math
import sys
from contextlib import ExitStack

import numpy as np

sys.path.insert(0, "/opt/trn_rl_repo")

import ml_dtypes  # noqa: E402

import concourse.bass as bass  # noqa: E402
import concourse.mybir as mybir  # noqa: E402
import concourse.tile as tile  # noqa: E402
from concourse.masks import make_identity  # noqa: E402

P = 128
BATCH_GATHER = False
F32 = mybir.dt.float32
BF16 = mybir.dt.bfloat16
I32 = mybir.dt.int32



# ---------------------------------------------------------------------------
# Workaround: the walrus build in this container accepts at most ONE attached
# sem wait per instruction ("Too many sync wait commands").  Tile attaches
# several (notably on its tail Drain).  Split extras into standalone
# EventSemaphore instructions (the exact form nc.sync.wait_ge() emits).
def _split_multiwaits_json(bir_json: bytes) -> bytes:
    import json as _json

    j = _json.loads(bir_json)
    n_new = 0
    for fn in j.get("functions", []):
        for bb in fn.get("blocks", []):
            out = []
            for ins in bb.get("instructions", []):
                si = ins.get("sync_info") or {}
                waits = si.get("on_wait") or []
                if len(waits) > 1:
                    for w in waits[:-1]:
                        n_new += 1
                        out.append(
                            dict(
                                debug=ins.get("debug", 0),
                                engine=ins["engine"],
                                ins=[],
                                outs=[],
                                name=f"{ins['name']}-hoistw{n_new}",
                                opcode="EventSemaphore",
                                sync_info=dict(on_update=[], on_wait=[w]),
                            )
                        )
                    si["on_wait"] = [waits[-1]]
                out.append(ins)
            bb["instructions"] = out
    return _json.dumps(j).encode()


def _install_compile_patch():
    from concourse import bass_utils as _bu

    if getattr(_bu, "_multiwait_patch", False):
        return
    orig = _bu.compile_bir_kernel

    def patched(bir_json, tmpdir, neff_name="file.neff"):
        return orig(_split_multiwaits_json(bir_json), tmpdir, neff_name)

    _bu.compile_bir_kernel = patched
    _bu._multiwait_patch = True
    try:
        from concourse import bass2jax as _b2j

        _b2j.compile_bir_kernel = patched
    except Exception:
        pass


class Cfg:
    def __init__(self, n_nodes, n_edges, n_cores, k_tiles):
        self.n_nodes = n_nodes
        self.n_edges = n_edges
        self.n_cores = n_cores
        self.k_tiles = k_tiles  # tiles of 128 edge-slots per node block
        self.n_blocks = math.ceil(n_nodes / P)  # real node blocks
        self.bpc = math.ceil(self.n_blocks / n_cores)  # blocks per core
        self.slots_per_block = k_tiles * P
        # chunk split of a block's K*128 slots into matmul-N-sized pieces
        self.chunks = []
        off = 0
        rem = self.slots_per_block
        while rem > 0:
            cs = min(512, rem)
            self.chunks.append((off, cs))
            off += cs
            rem -= cs


def preprocess(x, edge_index, edge_attr, cfg):
    """Sort/pad edges; build per-core device input arrays."""
    row = np.asarray(edge_index[0], dtype=np.int64)
    col = np.asarray(edge_index[1], dtype=np.int64)
    order = np.argsort(row, kind="stable")
    row_s = row[order]
    bounds = np.searchsorted(row_s, P * np.arange(cfg.n_blocks + 1))

    S = cfg.slots_per_block
    n_total_blocks = cfg.n_cores * cfg.bpc
    slot_eid = np.zeros((n_total_blocks, S), dtype=np.int64)
    slot_valid = np.zeros((n_total_blocks, S), dtype=bool)
    slot_rowlocal = np.full((n_total_blocks, S), -1.0, dtype=np.float32)
    slot_col = np.zeros((n_total_blocks, S), dtype=np.int32)
    for b in range(cfg.n_blocks):
        s, e = bounds[b], bounds[b + 1]
        n = e - s
        assert n <= S, f"block {b} has {n} edges > {S} slots; raise k_tiles"
        eids = order[s:e]
        slot_eid[b, :n] = eids
        slot_valid[b, :n] = True
        slot_rowlocal[b, :n] = (row[eids] - P * b).astype(np.float32)
        slot_col[b, :n] = col[eids].astype(np.int32)

    x_bf = x.astype(ml_dtypes.bfloat16)
    pad_rows = n_total_blocks * P - x_bf.shape[0]
    x_pad = np.concatenate(
        [x_bf, np.zeros((pad_rows, x.shape[1]), dtype=ml_dtypes.bfloat16)], axis=0
    )

    per_core = []
    for c in range(cfg.n_cores):
        blks = slice(c * cfg.bpc, (c + 1) * cfg.bpc)
        ea = np.zeros((cfg.bpc, S, x.shape[1]), dtype=ml_dtypes.bfloat16)
        eid_c = slot_eid[blks]
        val_c = slot_valid[blks]
        ea[val_c] = edge_attr[eid_c[val_c]].astype(ml_dtypes.bfloat16)
        ea_cm = np.ascontiguousarray(ea.transpose(0, 2, 1))
        ci = slot_col[blks].reshape(cfg.bpc, cfg.k_tiles, P).transpose(0, 2, 1)
        rl = slot_rowlocal[blks].reshape(cfg.bpc, cfg.k_tiles, P).transpose(0, 2, 1)
        per_core.append(
            dict(
                ea_cm=ea_cm,
                colidx=np.ascontiguousarray(ci),
                rowlocal=np.ascontiguousarray(rl),
                x_full=x_pad,
                x_slice=np.ascontiguousarray(
                    x_pad[c * cfg.bpc * P : (c + 1) * cfg.bpc * P]
                ),
            )
        )
    meta = dict(slot_eid=slot_eid, slot_valid=slot_valid, row=row)
    return per_core, meta


def make_weight_inputs(ew1, eb1, ew2, eb2, nw1, nb1, nw2, nb2):
    ew1 = np.asarray(ew1, np.float32)
    eb1 = np.asarray(eb1, np.float32)
    ew2 = np.asarray(ew2, np.float32)
    eb2 = np.asarray(eb2, np.float32)
    nw1 = np.asarray(nw1, np.float32)
    nb1 = np.asarray(nb1, np.float32)
    nw2 = np.asarray(nw2, np.float32)
    bf = lambda a: np.ascontiguousarray(a.astype(ml_dtypes.bfloat16))
    f32 = lambda a: np.ascontiguousarray(a.astype(np.float32))
    wfuse = ew2 @ nw1[128:256]  # new_edge_attr -> h3, fused through L2e
    bfuse = eb2 @ nw1[128:256]  # eb2's contribution to h3 preactivation
    return dict(
        w1a=bf(ew1[:128]),
        w1b=bf(ew1[128:256]),
        w1c=bf(ew1[256:384]),
        w2e=bf(ew2),
        w1na=bf(nw1[:128]),
        wfuse=bf(wfuse),
        w2n=bf(nw2),
        b1e=f32(eb1[:, None]),
        b1nf=f32((nb1 + bfuse)[:, None]),
        iota_f=f32(np.tile(np.arange(P, dtype=np.float32)[None, :], (P, 1))),
    )


WNAMES = ["w1a", "w1b", "w1c", "w2e", "w1na", "wfuse", "w2n"]
BSHAPES = dict(b1e=[P, 1], b1nf=[P, 1], iota_f=[P, P])


def build_program(cfg, sel_on_gpsimd=False, iters=1):
    CH = P
    S = cfg.slots_per_block
    K = cfg.k_tiles
    nc = bass.Bass(
        "TRN2", target_bir_lowering=False, debug=False, num_devices=cfg.n_cores
    )

    d_x_full = nc.declare_dram_parameter(
        "x_full", [cfg.n_cores * cfg.bpc * P, CH], BF16, isOutput=False
    )
    d_x_slice = nc.declare_dram_parameter(
        "x_slice", [cfg.bpc * P, CH], BF16, isOutput=False
    )
    d_ea = nc.declare_dram_parameter("ea_cm", [cfg.bpc, CH, S], BF16, isOutput=False)
    d_ci = nc.declare_dram_parameter("colidx", [cfg.bpc, P, K], I32, isOutput=False)
    d_rl = nc.declare_dram_parameter("rowlocal", [cfg.bpc, P, K], F32, isOutput=False)
    dW = {k: nc.declare_dram_parameter(k, [CH, CH], BF16, isOutput=False) for k in WNAMES}
    dB = {k: nc.declare_dram_parameter(k, v, F32, isOutput=False) for k, v in BSHAPES.items()}
    d_ne = nc.declare_dram_parameter("ne_out", [cfg.bpc, S, CH], F32, isOutput=True)
    d_seg = nc.declare_dram_parameter("seg_out", [cfg.bpc, P, CH], F32, isOutput=True)

    relu = mybir.ActivationFunctionType.Relu
    copyf = mybir.ActivationFunctionType.Copy

    with tile.TileContext(nc) as tc, ExitStack() as ctx:
        cpool = ctx.enter_context(tc.tile_pool(name="const", bufs=1))
        blkpool = ctx.enter_context(tc.tile_pool(name="blk", bufs=2))
        work = ctx.enter_context(tc.tile_pool(name="work", bufs=3))
        mpsum = ctx.enter_context(tc.tile_pool(name="mpsum", bufs=3, space="PSUM"))
        tpsum = ctx.enter_context(tc.tile_pool(name="tpsum", bufs=2, space="PSUM"))
        segpsum = ctx.enter_context(tc.tile_pool(name="segpsum", bufs=2, space="PSUM"))

        identb = cpool.tile([P, P], BF16)
        make_identity(nc, identb[:])

        W = {}
        for k in WNAMES:
            W[k] = cpool.tile([CH, CH], BF16, tag=f"W{k}", name=f"W_{k}")
            nc.sync.dma_start(out=W[k][:], in_=dW[k][:])
        B = {}
        for k, shp in BSHAPES.items():
            B[k] = cpool.tile(shp, F32, tag=f"B{k}", name=f"B_{k}")
            nc.sync.dma_start(out=B[k][:], in_=dB[k][:])

        for _rep in range(iters):
            _block_body(nc, tc, cfg, cpool, blkpool, work, mpsum, tpsum, segpsum,
                        identb, W, B, d_x_slice, d_ci, d_rl, d_ea, d_x_full,
                        d_ne, d_seg, sel_on_gpsimd)

    return nc


def _block_body(nc, tc, cfg, cpool, blkpool, work, mpsum, tpsum, segpsum,
                identb, W, B, d_x_slice, d_ci, d_rl, d_ea, d_x_full,
                d_ne, d_seg, sel_on_gpsimd):
        CH = P
        S = cfg.slots_per_block
        K = cfg.k_tiles
        relu = mybir.ActivationFunctionType.Relu
        copyf = mybir.ActivationFunctionType.Copy
        for lb in range(cfg.bpc):
            x_blk = blkpool.tile([P, CH], BF16, tag="x_blk")
            nc.sync.dma_start(out=x_blk[:], in_=d_x_slice[lb * P : (lb + 1) * P, :])
            ci_blk = blkpool.tile([P, K], I32, tag="ci")
            nc.sync.dma_start(out=ci_blk[:], in_=d_ci[lb])
            rl_blk = blkpool.tile([P, K], F32, tag="rl")
            nc.sync.dma_start(out=rl_blk[:], in_=d_rl[lb])
            ea_blk = blkpool.tile([CH, S], BF16, tag="ea")
            nc.sync.dma_start(out=ea_blk[:], in_=d_ea[lb])
            ne_stage = blkpool.tile([P, K * CH], F32, tag="ne_stage")

            xbt_ps = tpsum.tile([P, 512], BF16, tag="tpb")
            nc.tensor.transpose(out=xbt_ps[:, :CH], in_=x_blk[:], identity=identb[:])
            x_blkT = blkpool.tile([CH, P], BF16, tag="xbt")
            nc.vector.tensor_copy(x_blkT[:], xbt_ps[:, :CH])
            pb_ps = mpsum.tile([P, 512], F32, tag="mm")
            nc.tensor.matmul(
                out=pb_ps[:, :CH], lhsT=x_blkT[:], rhs=W["w1a"][:], start=True, stop=True
            )
            pb = blkpool.tile([P, CH], BF16, tag="pb")
            nc.vector.tensor_copy(pb[:], pb_ps[:, :CH])

            seg_ps = segpsum.tile([P, CH], F32, tag="seg")

            for (coff, cs) in cfg.chunks:
                nt = cs // P
                t0 = coff // P

                # batched indirect gather of x[col]: nt rows per partition
                xc_g = work.tile([P, 512], BF16, tag="xc_g")
                if BATCH_GATHER:
                    nc.gpsimd.indirect_dma_start(
                        out=xc_g[:, : nt * CH],
                        out_offset=None,
                        in_=d_x_full[:],
                        in_offset=bass.IndirectOffsetOnAxis(
                            ap=ci_blk[:, t0 : t0 + nt], axis=0
                        ),
                    )
                else:
                    for t in range(nt):
                        nc.gpsimd.indirect_dma_start(
                            out=xc_g[:, t * CH : (t + 1) * CH],
                            out_offset=None,
                            in_=d_x_full[:],
                            in_offset=bass.IndirectOffsetOnAxis(
                                ap=ci_blk[:, t0 + t : t0 + t + 1], axis=0
                            ),
                        )

                # one-hot selection, whole chunk at once: sel[p, t, n] = (rl[p,t]==n)
                sel_em = work.tile([P, 512], BF16, tag="sel_em")
                for t in range(nt):
                    nc.vector.tensor_tensor(
                        out=sel_em[:, t * P : (t + 1) * P],
                        in0=rl_blk[:, t0 + t : t0 + t + 1].to_broadcast([P, P]),
                        in1=B["iota_f"][:],
                        op=mybir.AluOpType.is_equal,
                    )

                xct_ps = tpsum.tile([P, 512], BF16, tag="tpb")
                selg_ps = tpsum.tile([P, 512], BF16, tag="tpb")
                for t in range(nt):
                    ts_ = slice(t * P, (t + 1) * P)
                    nc.tensor.transpose(
                        out=xct_ps[:, ts_], in_=xc_g[:, t * CH : (t + 1) * CH],
                        identity=identb[:],
                    )
                    nc.tensor.transpose(
                        out=selg_ps[:, ts_], in_=sel_em[:, ts_], identity=identb[:]
                    )
                xc_cm = work.tile([CH, 512], BF16, tag="xc_cm")
                nc.scalar.activation(out=xc_cm[:, :cs], in_=xct_ps[:, :cs], func=copyf)
                sel_g = work.tile([P, 512], BF16, tag="sel_g")
                nc.vector.tensor_copy(sel_g[:, :cs], selg_ps[:, :cs])

                # ---- edge MLP (channel-major) ----
                h1_ps = mpsum.tile([CH, 512], F32, tag="mm")
                nc.tensor.matmul(
                    out=h1_ps[:, :cs], lhsT=pb[:], rhs=sel_g[:, :cs],
                    start=True, stop=False,
                )
                nc.tensor.matmul(
                    out=h1_ps[:, :cs], lhsT=W["w1b"][:], rhs=xc_cm[:, :cs],
                    start=False, stop=False,
                )
                nc.tensor.matmul(
                    out=h1_ps[:, :cs], lhsT=W["w1c"][:], rhs=ea_blk[:, coff : coff + cs],
                    start=False, stop=True,
                )
                h1s = work.tile([CH, 512], BF16, tag="h1s")
                nc.scalar.activation(
                    out=h1s[:, :cs], in_=h1_ps[:, :cs], func=relu, bias=B["b1e"][:]
                )

                # edge-major new_edge_attr (bias applied on host)
                ne_ps = mpsum.tile([P, 512], F32, tag="mm")
                for t in range(nt):
                    ts_ = slice(t * P, (t + 1) * P)
                    nc.tensor.matmul(
                        out=ne_ps[:, ts_], lhsT=h1s[:, ts_], rhs=W["w2e"][:],
                        start=True, stop=True,
                    )
                nc.vector.tensor_copy(
                    ne_stage[:, coff : coff + cs], ne_ps[:, :cs]
                )

                # ---- node MLP ----
                h3_ps = mpsum.tile([CH, 512], F32, tag="mm")
                nc.tensor.matmul(
                    out=h3_ps[:, :cs], lhsT=W["w1na"][:], rhs=xc_cm[:, :cs],
                    start=True, stop=False,
                )
                nc.tensor.matmul(
                    out=h3_ps[:, :cs], lhsT=W["wfuse"][:], rhs=h1s[:, :cs],
                    start=False, stop=True,
                )
                h3s = work.tile([CH, 512], BF16, tag="h3s")
                nc.scalar.activation(
                    out=h3s[:, :cs], in_=h3_ps[:, :cs], func=relu, bias=B["b1nf"][:]
                )

                msg_ps = mpsum.tile([P, 512], F32, tag="mm")
                for t in range(nt):
                    ts_ = slice(t * P, (t + 1) * P)
                    nc.tensor.matmul(
                        out=msg_ps[:, ts_], lhsT=h3s[:, ts_], rhs=W["w2n"][:],
                        start=True, stop=True,
                    )
                msg_em = work.tile([P, 512], BF16, tag="msg_em")
                nc.vector.tensor_copy(msg_em[:, :cs], msg_ps[:, :cs])

                # ---- scatter: seg += sel^T @ msg over the block ----
                for t in range(nt):
                    ts_ = slice(t * P, (t + 1) * P)
                    nc.tensor.matmul(
                        out=seg_ps[:],
                        lhsT=sel_em[:, ts_],
                        rhs=msg_em[:, ts_],
                        start=(t0 + t == 0),
                        stop=(t0 + t == K - 1),
                        skip_group_check=True,
                    )

            nc.sync.dma_start(
                out=d_ne[lb].rearrange("(t p) c -> p t c", p=P),
                in_=ne_stage[:].rearrange("p (t c) -> p t c", t=K),
            )
            seg_sb = blkpool.tile([P, CH], F32, tag="seg_sb")
            nc.vector.tensor_copy(seg_sb[:], seg_ps[:])
            nc.sync.dma_start(out=d_seg[lb], in_=seg_sb[:])


def postprocess(results, meta, cfg, eb2, nb2):
    n_nodes, n_edges = cfg.n_nodes, cfg.n_edges
    ch = P
    row = meta["row"]
    cnt = np.bincount(row, minlength=n_nodes).astype(np.float32)
    denom = np.maximum(cnt, 1.0)
    seg = np.concatenate(
        [np.asarray(r["seg_out"], dtype=np.float32).reshape(-1, ch) for r in results],
        axis=0,
    )
    new_x = (seg[:n_nodes] + cnt[:, None] * np.asarray(nb2, np.float32)[None, :]) / denom[
        :, None
    ]
    new_ea = np.empty((n_edges, ch), dtype=np.float32)
    sv = meta["slot_valid"].reshape(cfg.n_cores, -1)
    se = meta["slot_eid"].reshape(cfg.n_cores, -1)
    for c, r in enumerate(results):
        ne = np.asarray(r["ne_out"], dtype=np.float32).reshape(-1, ch)
        v = sv[c]
        new_ea[se[c][v]] = ne[v]
    new_ea += np.asarray(eb2, np.float32)[None, :]
    return new_x.astype(np.float32), new_ea


def build_all(x, edge_index, edge_attr, ew1, eb1, ew2, eb2, nw1, nb1, nw2, nb2,
              n_cores=8, iters=1):
    x = np.asarray(x, dtype=np.float32)
    edge_attr = np.asarray(edge_attr, dtype=np.float32)
    n_nodes = x.shape[0]
    n_edges = edge_attr.shape[0]
    row = np.asarray(edge_index[0], dtype=np.int64)
    blk_cnt = np.bincount(row // P, minlength=math.ceil(n_nodes / P))
    k_tiles = max(1, int(math.ceil(blk_cnt.max() / P)))
    cfg = Cfg(n_nodes, n_edges, n_cores, k_tiles)

    per_core, meta = preprocess(x, edge_index, edge_attr, cfg)
    wdict = make_weight_inputs(ew1, eb1, ew2, eb2, nw1, nb1, nw2, nb2)
    in_maps = []
    for c in range(n_cores):
        m = dict(per_core[c])
        m.update(wdict)
        in_maps.append(m)
    nc = build_program(cfg, iters=iters)
    return nc, in_maps, meta, cfg


def kernel(x, edge_index, edge_attr, ew1, eb1, ew2, eb2, nw1, nb1, nw2, nb2):
    _install_compile_patch()
    from concourse.bass_utils import run_bass_kernel_spmd

    nc, in_maps, meta, cfg = build_all(
        x, edge_index, edge_attr, ew1, eb1, ew2, eb2, nw1, nb1, nw2, nb2, n_cores=8
    )
    res = run_bass_kernel_spmd(nc, in_maps, list(range(cfg.n_cores)))
    return postprocess(res.results, meta, cfg, eb2, nb2)
